# revision 15
# baseline (speedup 1.0000x reference)
"""Trainium2 Bass kernel for nn_ComplexAttention (B=4,H=8,T=2048,D=256).

Strategy
--------
* Shard the 32 (b,h) pairs across 8 NeuronCores, 4 per core (data parallel).
* Algebraic eliminations (host-side fp64 weight algebra):
    - scores = Re(<q, k>) = Re(x1^H (Wq^H Wk) x2): a single complex
      projection K2 = x2 @ W2^T with W2 = Wq^H Wk replaces BOTH the q and k
      projections; scores = x1_r.K2_r + x1_i.K2_i against the RAW q input.
      (k-bias only shifts scores per-q-row -> softmax invariant -> dropped;
      q-bias needs a per-k correction r_k, emitted only when bq != 0.)
    - out = (attn @ v) Wo^T + bo = attn @ (x2 Wvo^T) + bo_eff with
      Wvo = Wo Wv: the context matmul directly produces the output
      projection; the separate o-projection is eliminated entirely.
      bo_eff = bo + Wo bv (v bias folded, valid since softmax rows sum to 1).
* All matmuls bf16 on the PE array (fp32 PSUM). fp8 was evaluated in
  simulation and busts the 2e-2 tolerance (3-6e-2) on every candidate stage.
* Softmax denominators: DVE accumulates the 16 k-chunk partial sums
  (bf16), then ONE all-ones [128,128] matmul reduces across partitions and
  broadcasts -- replacing 16 per-chunk ones-matmuls (saves ~13us/core PE).
* 3-deep software pipeline per q-tile: P(i)=scores/exp/chunk-adds,
  C(i-1)=sums-mm/ctx/normalize/|.|, G(i-2)=gate-mm/sigmoid/mul/store.
  Emitting G two steps behind keeps the PE from stalling on the DVE/ACT
  magnitude chain before the gate matmul.
* ACT runs the transcendentals (Exp/Sqrt/Sigmoid); DVE does copies, bias
  adds, squares, chunk-sums; denominators via reciprocal_approx_fast.
"""

import numpy as np
import ml_dtypes

B, H, T, D = 4, 8, 2048, 256
NCORES = 8
BH = B * H
BH_PER_CORE = BH // NCORES  # 4
P = 128
DC = D // P       # 2 chunks of the feature dim
QT = 512          # q-tile width (matmul free dim / PSUM bank)
NQT = T // QT     # 4 q-tiles
NKC = T // P      # 16 k-chunks
NTT = T // P      # 16 t-tiles for v
EPS = 1e-8
SCALE = 1.0 / np.sqrt(D)

WNAMES = ["w2r", "w2i", "w2in", "wvr", "wvi", "wvin", "wg"]
NW = len(WNAMES)
BNAMES = ["bor", "boi", "bgn"]  # bgn = NEGATED gate bias (for sigmoid-via-exp)
NB = len(BNAMES)

BF16 = ml_dtypes.bfloat16

_BUILT = {}  # qbias-flag -> compiled module


def _emit_kernel(nc, tc, ctx, tens, has_qbias):
    import concourse.bass as bass
    from concourse import mybir

    f32 = mybir.dt.float32
    bf16 = mybir.dt.bfloat16
    AF = mybir.ActivationFunctionType

    consts = ctx.enter_context(tc.tile_pool(name="consts", bufs=1))
    inpool = ctx.enter_context(tc.tile_pool(name="inpool", bufs=1))
    x1pool = ctx.enter_context(tc.tile_pool(name="x1pool", bufs=2))
    qkpool = ctx.enter_context(tc.tile_pool(name="qkpool", bufs=1))
    vpool = ctx.enter_context(tc.tile_pool(name="vpool", bufs=2))
    attnpool = ctx.enter_context(tc.tile_pool(name="attnpool", bufs=2))
    accpool = ctx.enter_context(tc.tile_pool(name="accpool", bufs=2))
    outppool = ctx.enter_context(tc.tile_pool(name="outppool", bufs=2))
    m2pool = ctx.enter_context(tc.tile_pool(name="m2pool", bufs=2))
    smallpool = ctx.enter_context(tc.tile_pool(name="smallpool", bufs=2))
    rpool = ctx.enter_context(tc.tile_pool(name="rpool", bufs=2))
    ps_mm = ctx.enter_context(tc.tile_pool(name="ps_mm", bufs=4, space="PSUM"))
    ps_ctx = ctx.enter_context(tc.tile_pool(name="ps_ctx", bufs=2, space="PSUM"))
    ps_sums = ctx.enter_context(tc.tile_pool(name="ps_sums", bufs=2, space="PSUM"))

    # ---- constants: biases first (tiny), then K2 weights, then the rest ----
    bpack = consts.tile([P, NB * DC], f32, tag="bpack")
    nc.sync.dma_start(bpack[:], tens["bpack"][:])
    bias = {name: bpack[:, i * DC:(i + 1) * DC] for i, name in enumerate(BNAMES)}

    wpack = consts.tile([P, DC, NW * D], bf16, tag="wpack")
    wp_dram = tens["wpack"].rearrange("c p d -> p c d")
    nc.sync.dma_start(wpack[:, :, :3 * D], wp_dram[:, :, :3 * D])
    w = {name: wpack[:, :, i * D:(i + 1) * D] for i, name in enumerate(WNAMES)}

    def load_consts_rest():
        nc.sync.dma_start(wpack[:, :, 3 * D:], wp_dram[:, :, 3 * D:])

    if has_qbias:
        cpack = consts.tile([P, DC, 2], bf16, tag="cpack")
        nc.sync.dma_start(cpack[:], tens["cpack"].rearrange("c p d -> p c d"))

    ones_k = consts.tile([P, P], bf16, tag="ones_k")
    nc.vector.memset(ones_k[:], 1.0)
    eps_t = consts.tile([P, 1], f32, tag="eps")
    nc.vector.memset(eps_t[:], EPS)

    def proj_dmajor(dst, src_r, src_i, w1, w2):
        """dst[:, co, t] (bf16, d-major) = w1 @ src_r + w2 @ src_i."""
        for co in range(DC):
            for tt in range(NQT):
                ts = slice(tt * QT, (tt + 1) * QT)
                ps = ps_mm.tile([P, QT], f32, tag="mm")
                nc.tensor.matmul(ps, w1[:, 0, co * P:(co + 1) * P], src_r[:, 0, ts],
                                 start=True, stop=False)
                nc.tensor.matmul(ps, w1[:, 1, co * P:(co + 1) * P], src_r[:, 1, ts],
                                 start=False, stop=False)
                nc.tensor.matmul(ps, w2[:, 0, co * P:(co + 1) * P], src_i[:, 0, ts],
                                 start=False, stop=False)
                nc.tensor.matmul(ps, w2[:, 1, co * P:(co + 1) * P], src_i[:, 1, ts],
                                 start=False, stop=True)
                nc.vector.tensor_copy(dst[:, co, ts], ps)

    def load_inputs(bh, mid_fn=None):
        with nc.named_scope(f"load{bh}"):
            x2r = inpool.tile([P, DC, T], bf16, tag="x2r", name="x2r")
            x2i = inpool.tile([P, DC, T], bf16, tag="x2i", name="x2i")
            x1r = x1pool.tile([P, DC, T], bf16, tag="x1r", name="x1r")
            x1i = x1pool.tile([P, DC, T], bf16, tag="x1i", name="x1i")
            # quarter the transfers (r/i interleaved) so the first projection
            # matmuls start after ~1/4 of the kv data instead of all of it
            for tt in range(NQT):
                ts = slice(tt * QT, (tt + 1) * QT)
                for t, name in ((x2r, "xkv_r"), (x2i, "xkv_i")):
                    nc.sync.dma_start(
                        t[:, :, ts],
                        tens[name][bh].rearrange("c p t -> p c t")[:, :, ts])
            if mid_fn is not None:
                mid_fn()  # remaining constants queue behind the kv inputs
            for tt in range(NQT):
                ts = slice(tt * QT, (tt + 1) * QT)
                for t, name in ((x1r, "xq_r"), (x1i, "xq_i")):
                    nc.sync.dma_start(
                        t[:, :, ts],
                        tens[name][bh].rearrange("c p t -> p c t")[:, :, ts])
        return x2r, x2i, x1r, x1i

    def load_and_proj(bh, preloaded=None):
        x2r, x2i, x1r, x1i = preloaded if preloaded else load_inputs(bh)
        with nc.named_scope(f"proj{bh}"):
            k2r = qkpool.tile([P, DC, T], bf16, tag="k2r")
            k2i = qkpool.tile([P, DC, T], bf16, tag="k2i")
            proj_dmajor(k2r, x2r, x2i, w["w2r"], w["w2in"])
            proj_dmajor(k2i, x2i, x2r, w["w2r"], w["w2i"])

            # v' = x2 @ Wvo^T in t-major layout (the ctx matmul's lhsT layout)
            v_r = vpool.tile([P, NTT, D], bf16, tag="v_r")
            v_i = vpool.tile([P, NTT, D], bf16, tag="v_i")
            for tt in range(NTT):
                tsl = slice(tt * P, (tt + 1) * P)
                for dst, t1, w1, t2, w2 in (
                    (v_r, x2r, "wvr", x2i, "wvin"),
                    (v_i, x2i, "wvr", x2r, "wvi"),
                ):
                    ps = ps_mm.tile([P, QT], f32, tag="mm")
                    pv = ps[:, :D]
                    nc.tensor.matmul(pv, t1[:, 0, tsl], w[w1][:, 0, :],
                                     start=True, stop=False)
                    nc.tensor.matmul(pv, t1[:, 1, tsl], w[w1][:, 1, :],
                                     start=False, stop=False)
                    nc.tensor.matmul(pv, t2[:, 0, tsl], w[w2][:, 0, :],
                                     start=False, stop=False)
                    nc.tensor.matmul(pv, t2[:, 1, tsl], w[w2][:, 1, :],
                                     start=False, stop=True)
                    # ACT drains the v' PSUMs (GpSimd cannot read PSUM): DVE
                    # is ~90% busy during the projection phase with the K2
                    # drains alone, while ACT idles there
                    nc.scalar.activation(dst[:, tt, :], pv, AF.Copy)

            rT = None
            if has_qbias:
                # r_k = SCALE * Re(bq . conj(k_k)) per k-token, exp bias.
                rps = ps_mm.tile([P, QT], f32, tag="mm")[:, :NKC]
                for kc in range(NKC):
                    ksl = slice(kc * P, (kc + 1) * P)
                    o1 = rps[:, kc:kc + 1]
                    nc.tensor.matmul(o1, x2r[:, 0, ksl], cpack[:, 0, 0:1],
                                     start=True, stop=False)
                    nc.tensor.matmul(o1, x2r[:, 1, ksl], cpack[:, 1, 0:1],
                                     start=False, stop=False)
                    nc.tensor.matmul(o1, x2i[:, 0, ksl], cpack[:, 0, 1:2],
                                     start=False, stop=False)
                    nc.tensor.matmul(o1, x2i[:, 1, ksl], cpack[:, 1, 1:2],
                                     start=False, stop=True)
                rT = rpool.tile([P, NKC], f32, tag="rT")
                nc.vector.tensor_copy(rT[:], rps)
        return {"bh": bh, "k2r": k2r, "k2i": k2i, "v_r": v_r, "v_i": v_i,
                "x1r": x1r, "x1i": x1i, "rT": rT}

    def produce(st, qt):
        """scoresT -> exp -> DVE chunk-accumulated partial sums."""
        k2r, k2i = st["k2r"], st["k2i"]
        x1r, x1i = st["x1r"], st["x1i"]
        qsl = slice(qt * QT, (qt + 1) * QT)
        with nc.named_scope(f"attn{st['bh']}_{qt}"):
            attn = attnpool.tile([P, NKC, QT], bf16, tag="attn")
            acc = accpool.tile([P, QT], bf16, tag="acc")
            acc2 = accpool.tile([P, QT], bf16, tag="acc2")
            for kc in range(NKC):
                ksl = slice(kc * P, (kc + 1) * P)
                sc = ps_mm.tile([P, QT], f32, tag="mm")
                nc.tensor.matmul(sc, k2r[:, 0, ksl], x1r[:, 0, qsl],
                                 start=True, stop=False)
                nc.tensor.matmul(sc, k2r[:, 1, ksl], x1r[:, 1, qsl],
                                 start=False, stop=False)
                nc.tensor.matmul(sc, k2i[:, 0, ksl], x1i[:, 0, qsl],
                                 start=False, stop=False)
                nc.tensor.matmul(sc, k2i[:, 1, ksl], x1i[:, 1, qsl],
                                 start=False, stop=True)
                if st["rT"] is not None:
                    nc.scalar.activation(attn[:, kc, :], sc, AF.Exp,
                                         scale=SCALE,
                                         bias=st["rT"][:, kc:kc + 1])
                else:
                    nc.scalar.activation(attn[:, kc, :], sc, AF.Exp, scale=SCALE)
                # chunk partial sums accumulate on two engines in parallel
                # (GpSimd takes the even chunks, DVE the odd ones); one
                # ones-matmul later reduces across partitions. bf16 partials
                # cost ~0.1% on the denominators -- well under tolerance.
                if kc == 0:
                    nc.gpsimd.tensor_copy(acc[:], attn[:, 0, :])
                elif kc == 1:
                    nc.vector.tensor_copy(acc2[:], attn[:, 1, :])
                elif kc % 2 == 0:
                    nc.gpsimd.tensor_add(acc[:], acc[:], attn[:, kc, :])
                else:
                    nc.vector.tensor_add(acc2[:], acc2[:], attn[:, kc, :])
            nc.vector.tensor_add(acc2[:], acc2[:], acc[:])
        return {"attn": attn, "acc": acc2, "qt": qt}

    def consume_ctx(st, pr, qoff=0, qw=QT, sums_ps=None, tail=False):
        """sums-mm -> ctx (= output projection, Wo folded into v') ->
        normalize+bias -> |.|^2 -> mag, for columns [qoff, qoff+qw)."""
        qt = pr["qt"]
        attn = pr["attn"]
        v_r, v_i = st["v_r"], st["v_i"]
        csl = slice(qoff, qoff + qw)
        bh = st["bh"]
        with nc.named_scope(f"ctx{bh}_{qt}"):
            if sums_ps is None:
                sums_ps = ps_sums.tile([P, QT], f32, tag="sums")
                nc.tensor.matmul(sums_ps, ones_k[:], pr["acc"][:],
                                 start=True, stop=True)
            bc = smallpool.tile([P, QT], f32, tag="bc", name="bc")[:, :qw]
            # ~18-bit accurate, ~5x faster than nc.vector.reciprocal; sums are
            # softmax denominators (~1e2..1e4), far from the undefined edges
            nc.vector.reciprocal_approx_fast(bc[:], sums_ps[:, csl])

            out_r = outppool.tile([P, DC, QT], f32, tag="out_r", name="out_r")[:, :, :qw]
            out_i = outppool.tile([P, DC, QT], f32, tag="out_i", name="out_i")[:, :, :qw]
            m2r = m2pool.tile([P, DC, QT], f32, tag="m2r", name="m2r")[:, :, :qw]
            m2i = m2pool.tile([P, DC, QT], f32, tag="m2i", name="m2i")[:, :, :qw]
            for dst, m2, vsrc, b in ((out_r, m2r, v_r, "bor"),
                                     (out_i, m2i, v_i, "boi")):
                for c in range(DC):
                    cps = ps_ctx.tile([P, QT], f32, tag="ctx", name="ctx")[:, :qw]
                    for kc in range(NKC):
                        nc.tensor.matmul(cps, vsrc[:, kc, c * P:(c + 1) * P],
                                         attn[:, kc, csl],
                                         start=(kc == 0), stop=(kc == NKC - 1),
                                         skip_group_check=True)
                    nc.vector.tensor_mul(dst[:, c, :], cps, bc[:])
                    if tail:
                        # at the kernel tail ACT is idle: square there (with
                        # the bias folded in) to shorten the DVE chain
                        # before the gate matmul
                        nc.scalar.activation(m2[:, c, :], dst[:, c, :], AF.Square,
                                             bias=bias[b][:, c:c + 1])
                    nc.vector.tensor_scalar_add(dst[:, c, :], dst[:, c, :],
                                                bias[b][:, c:c + 1])

            if not tail:
                nc.vector.tensor_mul(m2r[:], out_r[:], out_r[:])
                nc.vector.tensor_mul(m2i[:], out_i[:], out_i[:])
            nc.vector.tensor_add(m2r[:], m2r[:], m2i[:])
            mag = smallpool.tile([P, DC, QT], bf16, tag="mag", name="mag")[:, :, :qw]
            # sqrt(x+eps) = exp(0.5*ln(x+eps)): Ln and Exp live in the SAME
            # ACT function table (natural_log_exp_and_others), so the kernel
            # never pays the 1.3us ACT_TABLE_LOAD that Sqrt would force.
            nc.scalar.activation(m2i[:], m2r[:], AF.Ln, bias=eps_t[:])
            nc.scalar.activation(mag[:], m2i[:], AF.Exp, scale=0.5)
        return {"qt": qt, "qoff": qoff, "qw": qw, "out_r": out_r,
                "out_i": out_i, "mag": mag, "sums_ps": sums_ps}

    def consume_gate(st, cr):
        """gate matmul -> sigmoid -> out*gate -> store."""
        qt, qoff, qw = cr["qt"], cr["qoff"], cr["qw"]
        out_r, out_i, mag = cr["out_r"], cr["out_i"], cr["mag"]
        bh = st["bh"]
        with nc.named_scope(f"gate{bh}_{qt}"):
            # sigmoid(z+bg) = 1/(1 + exp(-z-bg)): keeps ACT on the Exp table
            # (no Sigmoid table switch); the reciprocal runs on DVE.
            ge = smallpool.tile([P, DC, QT], f32, tag="ge", name="ge")[:, :, :qw]
            gate = smallpool.tile([P, DC, QT], f32, tag="gate", name="gate")[:, :, :qw]
            for go in range(DC):
                gps = ps_mm.tile([P, QT], f32, tag="mm", name="mm")[:, :qw]
                nc.tensor.matmul(gps, w["wg"][:, 0, go * P:(go + 1) * P],
                                 mag[:, 0, :], start=True, stop=False)
                nc.tensor.matmul(gps, w["wg"][:, 1, go * P:(go + 1) * P],
                                 mag[:, 1, :], start=False, stop=True)
                nc.scalar.activation(ge[:, go, :], gps, AF.Exp, scale=-1.0,
                                     bias=bias["bgn"][:, go:go + 1])
            nc.vector.tensor_scalar_add(ge[:], ge[:], 1.0)
            nc.vector.reciprocal_approx_fast(gate[:], ge[:])

            ob_r = smallpool.tile([P, DC, QT], bf16, tag="ob_r", name="ob_r")[:, :, :qw]
            ob_i = smallpool.tile([P, DC, QT], bf16, tag="ob_i", name="ob_i")[:, :, :qw]
            nc.vector.tensor_mul(ob_r[:], out_r[:], gate[:])
            nc.vector.tensor_mul(ob_i[:], out_i[:], gate[:])
            osl = slice(qt * QT + qoff, qt * QT + qoff + qw)
            for c in range(DC):
                nc.sync.dma_start(tens["yr"][bh, c, :, osl], ob_r[:, c, :])
                nc.sync.dma_start(tens["yi"][bh, c, :, osl], ob_i[:, c, :])

    # ---- 3-deep pipelined emission: P(i); C(i-1); G(i-2) -------------------
    preloaded0 = load_inputs(0, mid_fn=load_consts_rest)

    pend_c = None  # (state, produced) awaiting consume_ctx
    pend_g = None  # (state, ctx-result) awaiting consume_gate
    for bh in range(BH_PER_CORE):
        st = load_and_proj(bh, preloaded=preloaded0 if bh == 0 else None)
        for qt in range(NQT):
            pr = produce(st, qt)
            if pend_c is not None:
                cr = consume_ctx(pend_c[0], pend_c[1])
                if pend_g is not None:
                    consume_gate(pend_g[0], pend_g[1])
                pend_g = (pend_c[0], cr)
            pend_c = (st, pr)
    # tail: last tile in halves; tail=True spreads the epilogue over ACT+DVE
    lst, lpr = pend_c
    cr1 = consume_ctx(lst, lpr, qoff=0, qw=QT // 2, tail=True)
    if pend_g is not None:
        consume_gate(pend_g[0], pend_g[1])
    cr2 = consume_ctx(lst, lpr, qoff=QT // 2, qw=QT // 2,
                      sums_ps=cr1["sums_ps"], tail=True)
    consume_gate(lst, cr1)
    consume_gate(lst, cr2)


def _build(has_qbias):
    if has_qbias in _BUILT:
        return _BUILT[has_qbias]
    from contextlib import ExitStack
    import concourse.tile as tile
    from concourse import bacc, mybir

    f32 = mybir.dt.float32
    bf16 = mybir.dt.bfloat16

    nc = bacc.Bacc("TRN2", target_bir_lowering=False, debug=False,
                   num_devices=NCORES)

    tens = {}
    for name in ("xq_r", "xq_i", "xkv_r", "xkv_i"):
        tens[name] = nc.dram_tensor(name, [BH_PER_CORE, DC, P, T], bf16,
                                    kind="ExternalInput").ap()
    tens["wpack"] = nc.dram_tensor("wpack", [DC, P, NW * D], bf16,
                                   kind="ExternalInput").ap()
    tens["bpack"] = nc.dram_tensor("bpack", [P, NB * DC], f32,
                                   kind="ExternalInput").ap()
    if has_qbias:
        tens["cpack"] = nc.dram_tensor("cpack", [DC, P, 2], bf16,
                                       kind="ExternalInput").ap()
    for name in ("yr", "yi"):
        tens[name] = nc.dram_tensor(name, [BH_PER_CORE, DC, P, T], bf16,
                                    kind="ExternalOutput").ap()

    with tile.TileContext(nc) as tc:
        with ExitStack() as ctx:
            _emit_kernel(nc, tc, ctx, tens, has_qbias)

    nc.compile()
    _BUILT[has_qbias] = nc
    return nc


def _wT_pack(wm):
    """[D,D] weight -> transposed [di, do] -> [DC, P, D] bf16 (di chunked)."""
    return np.ascontiguousarray(
        np.asarray(wm, dtype=np.float64).T.reshape(DC, P, D).astype(BF16))


def _bias_pack(b):
    """[D] bias -> [P, DC] f32 (per-partition d-major layout)."""
    return np.ascontiguousarray(
        np.asarray(b, dtype=np.float64).reshape(DC, P).T.astype(np.float32))


def _x_pack(x):
    """[BH, T, D] fp32 -> [BH, DC, P, T] bf16 (d-major, transposed)."""
    xb = x.astype(BF16)
    return np.ascontiguousarray(xb.reshape(BH, T, DC, P).transpose(0, 2, 3, 1))


def kernel(**inputs):
    inputs = {k: np.asarray(v) for k, v in inputs.items()}

    # ---- host-side fp64 weight algebra ------------------------------------
    Wq = inputs["q_wr"].astype(np.float64) + 1j * inputs["q_wi"].astype(np.float64)
    Wk = inputs["k_wr"].astype(np.float64) + 1j * inputs["k_wi"].astype(np.float64)
    Wv = inputs["v_wr"].astype(np.float64) + 1j * inputs["v_wi"].astype(np.float64)
    Wo = inputs["o_wr"].astype(np.float64) + 1j * inputs["o_wi"].astype(np.float64)
    W2 = np.conj(Wq).T @ Wk          # scores = Re(x1^H W2 ... ): K2 = x2 @ W2^T
    Wvo = Wo @ Wv                    # out = attn @ (x2 @ Wvo^T) + bo_eff
    bv = inputs["v_br"].astype(np.float64) + 1j * inputs["v_bi"].astype(np.float64)
    bo = inputs["o_br"].astype(np.float64) + 1j * inputs["o_bi"].astype(np.float64)
    bo_eff = bo + Wo @ bv

    bq_r = inputs["q_br"].astype(np.float64)
    bq_i = inputs["q_bi"].astype(np.float64)
    has_qbias = bool(np.any(bq_r) or np.any(bq_i))

    nc = _build(has_qbias)
    from concourse.bass_utils import run_bass_kernel_spmd

    xq_r = _x_pack(inputs["q_in_r"].reshape(BH, T, D))
    xq_i = _x_pack(inputs["q_in_i"].reshape(BH, T, D))
    xkv_r = _x_pack(inputs["kv_in_r"].reshape(BH, T, D))
    xkv_i = _x_pack(inputs["kv_in_i"].reshape(BH, T, D))

    wmats = {"w2r": W2.real, "w2i": W2.imag, "w2in": -W2.imag,
             "wvr": Wvo.real, "wvi": Wvo.imag, "wvin": -Wvo.imag,
             "wg": inputs["gate_w"].astype(np.float64)}
    wpack = np.concatenate([_wT_pack(wmats[n]) for n in WNAMES], axis=-1)

    bmats = {"bor": bo_eff.real, "boi": bo_eff.imag,
             "bgn": -inputs["gate_b"].astype(np.float64)}
    bpack = np.concatenate([_bias_pack(bmats[n]) for n in BNAMES], axis=-1)

    consts = {"wpack": np.ascontiguousarray(wpack),
              "bpack": np.ascontiguousarray(bpack)}
    if has_qbias:
        # r_k = SCALE * (c_r . x2_r[k] + c_i . x2_i[k]) with c = bq conj(Wk)
        Wk_r = inputs["k_wr"].astype(np.float64)
        Wk_i = inputs["k_wi"].astype(np.float64)
        c_r = (Wk_r.T @ bq_r + Wk_i.T @ bq_i) * SCALE
        c_i = (Wk_r.T @ bq_i - Wk_i.T @ bq_r) * SCALE
        cpack = np.stack([c_r.reshape(DC, P), c_i.reshape(DC, P)],
                         axis=-1).astype(BF16)
        consts["cpack"] = np.ascontiguousarray(cpack)

    in_maps = []
    for c in range(NCORES):
        sl = slice(c * BH_PER_CORE, (c + 1) * BH_PER_CORE)
        m = dict(consts)
        m["xq_r"] = xq_r[sl]
        m["xq_i"] = xq_i[sl]
        m["xkv_r"] = xkv_r[sl]
        m["xkv_i"] = xkv_i[sl]
        in_maps.append(m)

    res = run_bass_kernel_spmd(nc, in_maps, core_ids=list(range(NCORES)))

    def unpack(name):
        full = np.concatenate([res.results[c][name] for c in range(NCORES)], axis=0)
        # [BH, DC, P, T] -> [BH, T, DC*P] -> [B, H, T, D]
        return np.ascontiguousarray(
            full.transpose(0, 3, 1, 2).reshape(B, H, T, D).astype(np.float32))

    return unpack("yr"), unpack("yi")


if __name__ == "__main__":
    # smoke test with random inputs
    rng = np.random.default_rng(0)
    fake = {}
    for nm in ("q_in_r", "q_in_i", "kv_in_r", "kv_in_i"):
        fake[nm] = rng.standard_normal((B, H, T, D), dtype=np.float32)
    for p in ("q", "k", "v", "o"):
        fake[f"{p}_wr"] = rng.standard_normal((D, D), dtype=np.float32) * 0.044
        fake[f"{p}_wi"] = rng.standard_normal((D, D), dtype=np.float32) * 0.044
        fake[f"{p}_br"] = np.zeros(D, np.float32)
        fake[f"{p}_bi"] = np.zeros(D, np.float32)
    fake["gate_w"] = rng.standard_normal((D, D), dtype=np.float32) * 0.044
    fake["gate_b"] = np.zeros(D, np.float32)
    yr, yi = kernel(**fake)
    print("OK", yr.shape, yi.shape, yr.dtype)


# revision 18
# speedup vs baseline: 1.1998x; 1.1998x over previous
"""Trainium2 Bass kernel for nn_ComplexAttention (B=4,H=8,T=2048,D=256).

Strategy
--------
* Shard the 32 (b,h) pairs across 8 NeuronCores, 4 per core (data parallel).
* Algebraic eliminations (host-side fp64 weight algebra):
    - scores = Re(<q, k>) = Re(x1^H (Wq^H Wk) x2): a single complex
      projection K2 = x2 @ W2^T with W2 = Wq^H Wk replaces BOTH the q and k
      projections; scores = x1_r.K2_r + x1_i.K2_i against the RAW q input.
      (k-bias only shifts scores per-q-row -> softmax invariant -> dropped;
      q-bias needs a per-k correction r_k, emitted only when bq != 0.)
    - out = (attn @ v) Wo^T + bo = attn @ (x2 Wvo^T) + bo_eff with
      Wvo = Wo Wv: the context matmul directly produces the output
      projection; the separate o-projection is eliminated entirely.
      bo_eff = bo + Wo bv (v bias folded, valid since softmax rows sum to 1).
* All matmuls bf16 on the PE array (fp32 PSUM). fp8 was evaluated in
  simulation and busts the 2e-2 tolerance (3-6e-2) on every candidate stage.
* Softmax denominators: DVE accumulates the 16 k-chunk partial sums
  (bf16), then ONE all-ones [128,128] matmul reduces across partitions and
  broadcasts -- replacing 16 per-chunk ones-matmuls (saves ~13us/core PE).
* 3-deep software pipeline per q-tile: P(i)=scores/exp/chunk-adds,
  C(i-1)=sums-mm/ctx/normalize/|.|, G(i-2)=gate-mm/sigmoid/mul/store.
  Emitting G two steps behind keeps the PE from stalling on the DVE/ACT
  magnitude chain before the gate matmul.
* ACT runs the transcendentals (Exp/Sqrt/Sigmoid); DVE does copies, bias
  adds, squares, chunk-sums; denominators via reciprocal_approx_fast.
"""

import numpy as np
import ml_dtypes

B, H, T, D = 4, 8, 2048, 256
NCORES = 8
BH = B * H
BH_PER_CORE = BH // NCORES  # 4
P = 128
DC = D // P       # 2 chunks of the feature dim
QT = 512          # q-tile width (matmul free dim / PSUM bank)
NQT = T // QT     # 4 q-tiles
NKC = T // P      # 16 k-chunks
NTT = T // P      # 16 t-tiles for v
EPS = 1e-8
SCALE = 1.0 / np.sqrt(D)

WNAMES = ["w2r", "w2i", "w2in", "wvr", "wvi", "wvin", "wg"]
NW = len(WNAMES)
BNAMES = ["bor", "boi", "bgn"]  # bgn = NEGATED gate bias (for sigmoid-via-exp)
NB = len(BNAMES)

BF16 = ml_dtypes.bfloat16

_BUILT = {}  # qbias-flag -> compiled module


def _emit_kernel(nc, tc, ctx, tens, has_qbias):
    import concourse.bass as bass
    from concourse import mybir

    f32 = mybir.dt.float32
    bf16 = mybir.dt.bfloat16
    AF = mybir.ActivationFunctionType

    consts = ctx.enter_context(tc.tile_pool(name="consts", bufs=1))
    inpool = ctx.enter_context(tc.tile_pool(name="inpool", bufs=1))
    x1pool = ctx.enter_context(tc.tile_pool(name="x1pool", bufs=2))
    qkpool = ctx.enter_context(tc.tile_pool(name="qkpool", bufs=1))
    vpool = ctx.enter_context(tc.tile_pool(name="vpool", bufs=2))
    attnpool = ctx.enter_context(tc.tile_pool(name="attnpool", bufs=2))
    accpool = ctx.enter_context(tc.tile_pool(name="accpool", bufs=2))
    outppool = ctx.enter_context(tc.tile_pool(name="outppool", bufs=2))
    m2pool = ctx.enter_context(tc.tile_pool(name="m2pool", bufs=2))
    smallpool = ctx.enter_context(tc.tile_pool(name="smallpool", bufs=2))
    rpool = ctx.enter_context(tc.tile_pool(name="rpool", bufs=2))
    ps_mm = ctx.enter_context(tc.tile_pool(name="ps_mm", bufs=5, space="PSUM"))
    ps_ctx = ctx.enter_context(tc.tile_pool(name="ps_ctx", bufs=2, space="PSUM"))
    ps_sums = ctx.enter_context(tc.tile_pool(name="ps_sums", bufs=1, space="PSUM"))

    # ---- constants: biases first (tiny), then K2 weights, then the rest ----
    bpack = consts.tile([P, NB * DC], f32, tag="bpack")
    nc.sync.dma_start(bpack[:], tens["bpack"][:])
    bias = {name: bpack[:, i * DC:(i + 1) * DC] for i, name in enumerate(BNAMES)}

    wpack = consts.tile([P, DC, NW * D], bf16, tag="wpack")
    wp_dram = tens["wpack"].rearrange("c p d -> p c d")
    nc.sync.dma_start(wpack[:, :, :3 * D], wp_dram[:, :, :3 * D])
    w = {name: wpack[:, :, i * D:(i + 1) * D] for i, name in enumerate(WNAMES)}

    def load_consts_rest():
        nc.sync.dma_start(wpack[:, :, 3 * D:], wp_dram[:, :, 3 * D:])

    if has_qbias:
        cpack = consts.tile([P, DC, 2], bf16, tag="cpack")
        nc.sync.dma_start(cpack[:], tens["cpack"].rearrange("c p d -> p c d"))

    ones_k = consts.tile([P, P], bf16, tag="ones_k")
    nc.vector.memset(ones_k[:], 1.0)
    eps_t = consts.tile([P, 1], f32, tag="eps")
    nc.vector.memset(eps_t[:], EPS)

    def proj_dmajor(dst, src_r, src_i, w1, w2):
        """dst[:, co, t] (bf16, d-major) = w1 @ src_r + w2 @ src_i."""
        for co in range(DC):
            for tt in range(NQT):
                ts = slice(tt * QT, (tt + 1) * QT)
                ps = ps_mm.tile([P, QT], f32, tag="mm")
                nc.tensor.matmul(ps, w1[:, 0, co * P:(co + 1) * P], src_r[:, 0, ts],
                                 start=True, stop=False)
                nc.tensor.matmul(ps, w1[:, 1, co * P:(co + 1) * P], src_r[:, 1, ts],
                                 start=False, stop=False)
                nc.tensor.matmul(ps, w2[:, 0, co * P:(co + 1) * P], src_i[:, 0, ts],
                                 start=False, stop=False)
                nc.tensor.matmul(ps, w2[:, 1, co * P:(co + 1) * P], src_i[:, 1, ts],
                                 start=False, stop=True)
                nc.vector.tensor_copy(dst[:, co, ts], ps)

    def load_inputs(bh, mid_fn=None):
        with nc.named_scope(f"load{bh}"):
            x2r = inpool.tile([P, DC, T], bf16, tag="x2r", name="x2r")
            x2i = inpool.tile([P, DC, T], bf16, tag="x2i", name="x2i")
            x1r = x1pool.tile([P, DC, T], bf16, tag="x1r", name="x1r")
            x1i = x1pool.tile([P, DC, T], bf16, tag="x1i", name="x1i")
            # quarter the transfers (r/i interleaved) so the first projection
            # matmuls start after ~1/4 of the kv data instead of all of it
            for tt in range(NQT):
                ts = slice(tt * QT, (tt + 1) * QT)
                for t, name in ((x2r, "xkv_r"), (x2i, "xkv_i")):
                    nc.sync.dma_start(
                        t[:, :, ts],
                        tens[name][bh].rearrange("c p t -> p c t")[:, :, ts])
            if mid_fn is not None:
                mid_fn()  # remaining constants queue behind the kv inputs
            for tt in range(NQT):
                ts = slice(tt * QT, (tt + 1) * QT)
                for t, name in ((x1r, "xq_r"), (x1i, "xq_i")):
                    nc.sync.dma_start(
                        t[:, :, ts],
                        tens[name][bh].rearrange("c p t -> p c t")[:, :, ts])
        return x2r, x2i, x1r, x1i

    def load_and_proj(bh, preloaded=None):
        x2r, x2i, x1r, x1i = preloaded if preloaded else load_inputs(bh)
        with nc.named_scope(f"proj{bh}"):
            k2r = qkpool.tile([P, DC, T], bf16, tag="k2r")
            k2i = qkpool.tile([P, DC, T], bf16, tag="k2i")
            proj_dmajor(k2r, x2r, x2i, w["w2r"], w["w2in"])
            proj_dmajor(k2i, x2i, x2r, w["w2r"], w["w2i"])

            # v' = x2 @ Wvo^T in t-major layout (the ctx matmul's lhsT layout)
            v_r = vpool.tile([P, NTT, D], bf16, tag="v_r")
            v_i = vpool.tile([P, NTT, D], bf16, tag="v_i")
            for tt in range(NTT):
                tsl = slice(tt * P, (tt + 1) * P)
                for dst, t1, w1, t2, w2 in (
                    (v_r, x2r, "wvr", x2i, "wvin"),
                    (v_i, x2i, "wvr", x2r, "wvi"),
                ):
                    ps = ps_mm.tile([P, QT], f32, tag="mm")
                    pv = ps[:, :D]
                    nc.tensor.matmul(pv, t1[:, 0, tsl], w[w1][:, 0, :],
                                     start=True, stop=False)
                    nc.tensor.matmul(pv, t1[:, 1, tsl], w[w1][:, 1, :],
                                     start=False, stop=False)
                    nc.tensor.matmul(pv, t2[:, 0, tsl], w[w2][:, 0, :],
                                     start=False, stop=False)
                    nc.tensor.matmul(pv, t2[:, 1, tsl], w[w2][:, 1, :],
                                     start=False, stop=True)
                    # DVE drains the v' PSUMs. Putting these on ACT builds a
                    # ~12us ACT backlog per bh that delays every exp stream
                    # (and the PSUM frees the scores matmuls wait on).
                    nc.vector.tensor_copy(dst[:, tt, :], pv)

            rT = None
            if has_qbias:
                # r_k = SCALE * Re(bq . conj(k_k)) per k-token, exp bias.
                rps = ps_mm.tile([P, QT], f32, tag="mm")[:, :NKC]
                for kc in range(NKC):
                    ksl = slice(kc * P, (kc + 1) * P)
                    o1 = rps[:, kc:kc + 1]
                    nc.tensor.matmul(o1, x2r[:, 0, ksl], cpack[:, 0, 0:1],
                                     start=True, stop=False)
                    nc.tensor.matmul(o1, x2r[:, 1, ksl], cpack[:, 1, 0:1],
                                     start=False, stop=False)
                    nc.tensor.matmul(o1, x2i[:, 0, ksl], cpack[:, 0, 1:2],
                                     start=False, stop=False)
                    nc.tensor.matmul(o1, x2i[:, 1, ksl], cpack[:, 1, 1:2],
                                     start=False, stop=True)
                rT = rpool.tile([P, NKC], f32, tag="rT")
                nc.vector.tensor_copy(rT[:], rps)
        return {"bh": bh, "k2r": k2r, "k2i": k2i, "v_r": v_r, "v_i": v_i,
                "x1r": x1r, "x1i": x1i, "rT": rT}

    def produce(st, qt):
        """scoresT -> exp -> DVE chunk-accumulated partial sums."""
        k2r, k2i = st["k2r"], st["k2i"]
        x1r, x1i = st["x1r"], st["x1i"]
        qsl = slice(qt * QT, (qt + 1) * QT)
        with nc.named_scope(f"attn{st['bh']}_{qt}"):
            attn = attnpool.tile([P, NKC, QT], bf16, tag="attn")
            acc = accpool.tile([P, QT], bf16, tag="acc")
            acc2 = accpool.tile([P, QT], bf16, tag="acc2")
            for kc in range(NKC):
                ksl = slice(kc * P, (kc + 1) * P)
                sc = ps_mm.tile([P, QT], f32, tag="mm")
                nc.tensor.matmul(sc, k2r[:, 0, ksl], x1r[:, 0, qsl],
                                 start=True, stop=False)
                nc.tensor.matmul(sc, k2r[:, 1, ksl], x1r[:, 1, qsl],
                                 start=False, stop=False)
                nc.tensor.matmul(sc, k2i[:, 0, ksl], x1i[:, 0, qsl],
                                 start=False, stop=False)
                nc.tensor.matmul(sc, k2i[:, 1, ksl], x1i[:, 1, qsl],
                                 start=False, stop=True)
                if st["rT"] is not None:
                    nc.scalar.activation(attn[:, kc, :], sc, AF.Exp,
                                         scale=SCALE,
                                         bias=st["rT"][:, kc:kc + 1])
                else:
                    nc.scalar.activation(attn[:, kc, :], sc, AF.Exp, scale=SCALE)
                # chunk partial sums accumulate on two engines in parallel
                # (GpSimd takes the even chunks, DVE the odd ones); one
                # ones-matmul later reduces across partitions. bf16 partials
                # cost ~0.1% on the denominators -- well under tolerance.
                if kc == 0:
                    nc.gpsimd.tensor_copy(acc[:], attn[:, 0, :])
                elif kc == 1:
                    nc.vector.tensor_copy(acc2[:], attn[:, 1, :])
                elif kc % 2 == 0:
                    nc.gpsimd.tensor_add(acc[:], acc[:], attn[:, kc, :])
                else:
                    nc.vector.tensor_add(acc2[:], acc2[:], attn[:, kc, :])
            nc.vector.tensor_add(acc2[:], acc2[:], acc[:])
        return {"attn": attn, "acc": acc2, "qt": qt}

    def consume_ctx(st, pr, qoff=0, qw=QT, sums_ps=None, tail=False):
        """sums-mm -> ctx (= output projection, Wo folded into v') ->
        normalize+bias -> |.|^2 -> mag, for columns [qoff, qoff+qw)."""
        qt = pr["qt"]
        attn = pr["attn"]
        v_r, v_i = st["v_r"], st["v_i"]
        csl = slice(qoff, qoff + qw)
        bh = st["bh"]
        with nc.named_scope(f"ctx{bh}_{qt}"):
            if sums_ps is None:
                sums_ps = ps_sums.tile([P, QT], f32, tag="sums")
                nc.tensor.matmul(sums_ps, ones_k[:], pr["acc"][:],
                                 start=True, stop=True)
            bc = smallpool.tile([P, QT], f32, tag="bc", name="bc")[:, :qw]
            # ~18-bit accurate, ~5x faster than nc.vector.reciprocal; sums are
            # softmax denominators (~1e2..1e4), far from the undefined edges
            nc.vector.reciprocal_approx_fast(bc[:], sums_ps[:, csl])

            out_r = outppool.tile([P, DC, QT], f32, tag="out_r", name="out_r")[:, :, :qw]
            out_i = outppool.tile([P, DC, QT], f32, tag="out_i", name="out_i")[:, :, :qw]
            m2r = m2pool.tile([P, DC, QT], f32, tag="m2r", name="m2r")[:, :, :qw]
            m2i = m2pool.tile([P, DC, QT], f32, tag="m2i", name="m2i")[:, :, :qw]
            for dst, m2, vsrc, b in ((out_r, m2r, v_r, "bor"),
                                     (out_i, m2i, v_i, "boi")):
                for c in range(DC):
                    cps = ps_ctx.tile([P, QT], f32, tag="ctx", name="ctx")[:, :qw]
                    for kc in range(NKC):
                        nc.tensor.matmul(cps, vsrc[:, kc, c * P:(c + 1) * P],
                                         attn[:, kc, csl],
                                         start=(kc == 0), stop=(kc == NKC - 1),
                                         skip_group_check=True)
                    nc.vector.tensor_mul(dst[:, c, :], cps, bc[:])
                    if tail:
                        # at the kernel tail ACT is idle: square there (with
                        # the bias folded in) to shorten the DVE chain
                        # before the gate matmul
                        nc.scalar.activation(m2[:, c, :], dst[:, c, :], AF.Square,
                                             bias=bias[b][:, c:c + 1])
                    nc.vector.tensor_scalar_add(dst[:, c, :], dst[:, c, :],
                                                bias[b][:, c:c + 1])

            if not tail:
                nc.vector.tensor_mul(m2r[:], out_r[:], out_r[:])
                nc.vector.tensor_mul(m2i[:], out_i[:], out_i[:])
            nc.vector.tensor_add(m2r[:], m2r[:], m2i[:])
            mag = smallpool.tile([P, DC, QT], bf16, tag="mag", name="mag")[:, :, :qw]
            nc.scalar.activation(mag[:], m2r[:], AF.Sqrt, bias=eps_t[:])
        return {"qt": qt, "qoff": qoff, "qw": qw, "out_r": out_r,
                "out_i": out_i, "mag": mag, "sums_ps": sums_ps}

    def consume_gate(st, cr):
        """gate matmul -> sigmoid -> out*gate -> store."""
        qt, qoff, qw = cr["qt"], cr["qoff"], cr["qw"]
        out_r, out_i, mag = cr["out_r"], cr["out_i"], cr["mag"]
        bh = st["bh"]
        with nc.named_scope(f"gate{bh}_{qt}"):
            # sigmoid(z+bg) = 1/(1 + exp(-z-bg)): keeps ACT on the Exp table
            # (no Sigmoid table switch); the reciprocal runs on DVE.
            ge = smallpool.tile([P, DC, QT], f32, tag="ge", name="ge")[:, :, :qw]
            gate = smallpool.tile([P, DC, QT], f32, tag="gate", name="gate")[:, :, :qw]
            for go in range(DC):
                gps = ps_mm.tile([P, QT], f32, tag="mm", name="mm")[:, :qw]
                nc.tensor.matmul(gps, w["wg"][:, 0, go * P:(go + 1) * P],
                                 mag[:, 0, :], start=True, stop=False)
                nc.tensor.matmul(gps, w["wg"][:, 1, go * P:(go + 1) * P],
                                 mag[:, 1, :], start=False, stop=True)
                nc.scalar.activation(ge[:, go, :], gps, AF.Exp, scale=-1.0,
                                     bias=bias["bgn"][:, go:go + 1])
            nc.vector.tensor_scalar_add(ge[:], ge[:], 1.0)
            nc.vector.reciprocal_approx_fast(gate[:], ge[:])

            ob_r = smallpool.tile([P, DC, QT], bf16, tag="ob_r", name="ob_r")[:, :, :qw]
            ob_i = smallpool.tile([P, DC, QT], bf16, tag="ob_i", name="ob_i")[:, :, :qw]
            nc.vector.tensor_mul(ob_r[:], out_r[:], gate[:])
            nc.vector.tensor_mul(ob_i[:], out_i[:], gate[:])
            osl = slice(qt * QT + qoff, qt * QT + qoff + qw)
            for c in range(DC):
                nc.sync.dma_start(tens["yr"][bh, c, :, osl], ob_r[:, c, :])
                nc.sync.dma_start(tens["yi"][bh, c, :, osl], ob_i[:, c, :])

    # ---- 3-deep pipelined emission: P(i); C(i-1); G(i-2) -------------------
    preloaded0 = load_inputs(0, mid_fn=load_consts_rest)

    pend_c = None  # (state, produced) awaiting consume_ctx
    pend_g = None  # (state, ctx-result) awaiting consume_gate
    for bh in range(BH_PER_CORE):
        st = load_and_proj(bh, preloaded=preloaded0 if bh == 0 else None)
        for qt in range(NQT):
            pr = produce(st, qt)
            if pend_c is not None:
                cr = consume_ctx(pend_c[0], pend_c[1])
                if pend_g is not None:
                    consume_gate(pend_g[0], pend_g[1])
                pend_g = (pend_c[0], cr)
            pend_c = (st, pr)
    # tail: last tile in halves; tail=True spreads the epilogue over ACT+DVE
    lst, lpr = pend_c
    cr1 = consume_ctx(lst, lpr, qoff=0, qw=QT // 2, tail=True)
    if pend_g is not None:
        consume_gate(pend_g[0], pend_g[1])
    cr2 = consume_ctx(lst, lpr, qoff=QT // 2, qw=QT // 2,
                      sums_ps=cr1["sums_ps"], tail=True)
    consume_gate(lst, cr1)
    consume_gate(lst, cr2)


def _build(has_qbias):
    if has_qbias in _BUILT:
        return _BUILT[has_qbias]
    from contextlib import ExitStack
    import concourse.tile as tile
    from concourse import bacc, mybir

    f32 = mybir.dt.float32
    bf16 = mybir.dt.bfloat16

    nc = bacc.Bacc("TRN2", target_bir_lowering=False, debug=False,
                   num_devices=NCORES)

    tens = {}
    for name in ("xq_r", "xq_i", "xkv_r", "xkv_i"):
        tens[name] = nc.dram_tensor(name, [BH_PER_CORE, DC, P, T], bf16,
                                    kind="ExternalInput").ap()
    tens["wpack"] = nc.dram_tensor("wpack", [DC, P, NW * D], bf16,
                                   kind="ExternalInput").ap()
    tens["bpack"] = nc.dram_tensor("bpack", [P, NB * DC], f32,
                                   kind="ExternalInput").ap()
    if has_qbias:
        tens["cpack"] = nc.dram_tensor("cpack", [DC, P, 2], bf16,
                                       kind="ExternalInput").ap()
    for name in ("yr", "yi"):
        tens[name] = nc.dram_tensor(name, [BH_PER_CORE, DC, P, T], bf16,
                                    kind="ExternalOutput").ap()

    with tile.TileContext(nc) as tc:
        with ExitStack() as ctx:
            _emit_kernel(nc, tc, ctx, tens, has_qbias)

    nc.compile()
    _BUILT[has_qbias] = nc
    return nc


def _wT_pack(wm):
    """[D,D] weight -> transposed [di, do] -> [DC, P, D] bf16 (di chunked)."""
    return np.ascontiguousarray(
        np.asarray(wm, dtype=np.float64).T.reshape(DC, P, D).astype(BF16))


def _bias_pack(b):
    """[D] bias -> [P, DC] f32 (per-partition d-major layout)."""
    return np.ascontiguousarray(
        np.asarray(b, dtype=np.float64).reshape(DC, P).T.astype(np.float32))


def _x_pack(x):
    """[BH, T, D] fp32 -> [BH, DC, P, T] bf16 (d-major, transposed)."""
    xb = x.astype(BF16)
    return np.ascontiguousarray(xb.reshape(BH, T, DC, P).transpose(0, 2, 3, 1))


def kernel(**inputs):
    inputs = {k: np.asarray(v) for k, v in inputs.items()}

    # ---- host-side fp64 weight algebra ------------------------------------
    Wq = inputs["q_wr"].astype(np.float64) + 1j * inputs["q_wi"].astype(np.float64)
    Wk = inputs["k_wr"].astype(np.float64) + 1j * inputs["k_wi"].astype(np.float64)
    Wv = inputs["v_wr"].astype(np.float64) + 1j * inputs["v_wi"].astype(np.float64)
    Wo = inputs["o_wr"].astype(np.float64) + 1j * inputs["o_wi"].astype(np.float64)
    W2 = np.conj(Wq).T @ Wk          # scores = Re(x1^H W2 ... ): K2 = x2 @ W2^T
    Wvo = Wo @ Wv                    # out = attn @ (x2 @ Wvo^T) + bo_eff
    bv = inputs["v_br"].astype(np.float64) + 1j * inputs["v_bi"].astype(np.float64)
    bo = inputs["o_br"].astype(np.float64) + 1j * inputs["o_bi"].astype(np.float64)
    bo_eff = bo + Wo @ bv

    bq_r = inputs["q_br"].astype(np.float64)
    bq_i = inputs["q_bi"].astype(np.float64)
    has_qbias = bool(np.any(bq_r) or np.any(bq_i))

    nc = _build(has_qbias)
    from concourse.bass_utils import run_bass_kernel_spmd

    xq_r = _x_pack(inputs["q_in_r"].reshape(BH, T, D))
    xq_i = _x_pack(inputs["q_in_i"].reshape(BH, T, D))
    xkv_r = _x_pack(inputs["kv_in_r"].reshape(BH, T, D))
    xkv_i = _x_pack(inputs["kv_in_i"].reshape(BH, T, D))

    wmats = {"w2r": W2.real, "w2i": W2.imag, "w2in": -W2.imag,
             "wvr": Wvo.real, "wvi": Wvo.imag, "wvin": -Wvo.imag,
             "wg": inputs["gate_w"].astype(np.float64)}
    wpack = np.concatenate([_wT_pack(wmats[n]) for n in WNAMES], axis=-1)

    bmats = {"bor": bo_eff.real, "boi": bo_eff.imag,
             "bgn": -inputs["gate_b"].astype(np.float64)}
    bpack = np.concatenate([_bias_pack(bmats[n]) for n in BNAMES], axis=-1)

    consts = {"wpack": np.ascontiguousarray(wpack),
              "bpack": np.ascontiguousarray(bpack)}
    if has_qbias:
        # r_k = SCALE * (c_r . x2_r[k] + c_i . x2_i[k]) with c = bq conj(Wk)
        Wk_r = inputs["k_wr"].astype(np.float64)
        Wk_i = inputs["k_wi"].astype(np.float64)
        c_r = (Wk_r.T @ bq_r + Wk_i.T @ bq_i) * SCALE
        c_i = (Wk_r.T @ bq_i - Wk_i.T @ bq_r) * SCALE
        cpack = np.stack([c_r.reshape(DC, P), c_i.reshape(DC, P)],
                         axis=-1).astype(BF16)
        consts["cpack"] = np.ascontiguousarray(cpack)

    in_maps = []
    for c in range(NCORES):
        sl = slice(c * BH_PER_CORE, (c + 1) * BH_PER_CORE)
        m = dict(consts)
        m["xq_r"] = xq_r[sl]
        m["xq_i"] = xq_i[sl]
        m["xkv_r"] = xkv_r[sl]
        m["xkv_i"] = xkv_i[sl]
        in_maps.append(m)

    res = run_bass_kernel_spmd(nc, in_maps, core_ids=list(range(NCORES)))

    def unpack(name):
        full = np.concatenate([res.results[c][name] for c in range(NCORES)], axis=0)
        # [BH, DC, P, T] -> [BH, T, DC*P] -> [B, H, T, D]
        return np.ascontiguousarray(
            full.transpose(0, 3, 1, 2).reshape(B, H, T, D).astype(np.float32))

    return unpack("yr"), unpack("yi")


if __name__ == "__main__":
    # smoke test with random inputs
    rng = np.random.default_rng(0)
    fake = {}
    for nm in ("q_in_r", "q_in_i", "kv_in_r", "kv_in_i"):
        fake[nm] = rng.standard_normal((B, H, T, D), dtype=np.float32)
    for p in ("q", "k", "v", "o"):
        fake[f"{p}_wr"] = rng.standard_normal((D, D), dtype=np.float32) * 0.044
        fake[f"{p}_wi"] = rng.standard_normal((D, D), dtype=np.float32) * 0.044
        fake[f"{p}_br"] = np.zeros(D, np.float32)
        fake[f"{p}_bi"] = np.zeros(D, np.float32)
    fake["gate_w"] = rng.standard_normal((D, D), dtype=np.float32) * 0.044
    fake["gate_b"] = np.zeros(D, np.float32)
    yr, yi = kernel(**fake)
    print("OK", yr.shape, yi.shape, yr.dtype)
